# revision 2
# baseline (speedup 1.0000x reference)
"""Trainium2 Bass kernel for top-2 MoE (nn_ExpertMemory).

Model (reference semantics):
    logits = x @ gate_w + gate_b          # (N, E)
    probs  = softmax(logits)
    gates, idx = top_k(probs, 2)
    out[n] = sum_k gates[n,k] * (relu(x[n] @ w1[e] + b1[e]) @ w2[e] + b2[e]),
             e = idx[n,k]
(The reference runs every expert densely, but combine weights are zero off
the top-2, so routed computation is mathematically identical.)

Strategy: data-parallel over tokens across 8 NeuronCores (1024 tokens each).
Each core, fully on device:
  1. gate matmul + softmax + top-2 (max8/max_index) on its tokens
  2. expert-sorted slot assignment via triangular-matmul prefix sums
  3. token->slot metadata staged through a small DRAM scatter (indirect DMA)
  4. per-expert MLP in a C-major (transposed) domain: activations gathered
     along the free axis with gpsimd ap_gather, fp32r matmuls at full PE rate
  5. gate scaling folded into the expert-output write; combine = two
     free-axis gathers + add; output written C-major, host transposes back.
Slot capacities are specialized per run from a host-side replica of the
routing (inputs only), with margin; the device computes everything itself.

Measured (8 cores, axon trn2, in-NEFF repeat timing): ~1.19 ms per
invocation. HW profiling showed gpsimd ap_gather costs ~24 ns per gathered
column ([128,1024] f32 gather = ~24.5 us, ~7x the cost model), so the
gpsimd engine (~0.85 ms of gather work: dispatch 18K cols, combine 16K,
gates 2K) is the wall-clock bottleneck, ahead of PE (~0.45 ms of matmul).
Next optimization: replace free-axis gathers with indirect-DMA row
gathers (token-major) + PE transposes into the C-major domain, and the
combine with PE transposes + indirect-DMA row scatter into rank-split
buffers; that moves ~0.85 ms of gpsimd work to DMA engines (~50 us) and
PE (~90 us), making the kernel PE-bound at roughly 0.4-0.5 ms.
"""

import numpy as np
from contextlib import ExitStack

import concourse.bass as bass
import concourse.tile as tile
import concourse.mybir as mybir
from concourse import bacc

dt = mybir.dt
AF = mybir.ActivationFunctionType
ALU = mybir.AluOpType
AX = mybir.AxisListType

P = 128

# problem constants
B, T, C, E, H, TOPK = 4, 2048, 1024, 8, 2048, 2
NCORE = 8
NTOK = B * T // NCORE  # tokens per core


def _mlp_tiles(s):
    """Split a slot range of size s into moving-dim tiles <= 512, preferring
    every tile >= 256 (fp32r full-rate threshold)."""
    out = []
    off = 0
    rem = s
    while rem > 0:
        if rem > 512:
            if rem < 768:  # split near-evenly, both >= 256
                w = (rem // 2 + 15) // 16 * 16
            else:
                w = 512
        else:
            w = rem
        out.append((off, w))
        off += w
        rem -= w
    return out




def build_program(S, ntok=NTOK, c=C, h=H, e=E, level=9, repeat=1):
    nc = _build(S, ntok=ntok, c=c, h=h, e=e, level=level, repeat=repeat)
    nc.compile()
    return nc


def _build(S, ntok=NTOK, c=C, h=H, e=E, level=9, repeat=1):
    """Build the SPMD Tile program. S: per-expert slot capacities
    (multiples of 16, sum multiple of 128)."""
    TCH = ntok // P   # token chunks
    CK = c // P       # contraction chunks for layer 1 / output chunks
    HK = h // P       # hidden chunks
    NH = ntok // 512  # 512-wide token halves
    NSLOT = int(sum(S))
    base = np.concatenate([[0], np.cumsum(S)]).astype(np.int64)
    assert NSLOT % P == 0 and all(s % 16 == 0 for s in S)
    assert NSLOT <= 32000  # int16 gather indices

    nc = bacc.Bacc("TRN2", target_bir_lowering=False, debug=False)

    f32, bf16 = dt.float32, dt.bfloat16
    xT = nc.dram_tensor("xT", [c, ntok], f32, kind="ExternalInput").ap()
    gw = nc.dram_tensor("gw", [c, e], f32, kind="ExternalInput").ap()
    gb = nc.dram_tensor("gb", [e, 1], f32, kind="ExternalInput").ap()
    w1 = nc.dram_tensor("w1", [e, HK, P, CK * P], bf16,
                        kind="ExternalInput").ap()
    b1 = nc.dram_tensor("b1", [e, h, 1], f32, kind="ExternalInput").ap()
    w2 = nc.dram_tensor("w2", [e, h, c], bf16, kind="ExternalInput").ap()
    b2 = nc.dram_tensor("b2", [e, c, 1], f32, kind="ExternalInput").ap()
    tri = nc.dram_tensor("tri", [P, P], f32, kind="ExternalInput").ap()
    id8 = nc.dram_tensor("id8", [e, e], f32, kind="ExternalInput").ap()
    ebase = nc.dram_tensor("ebase", [1, e], f32, kind="ExternalInput").ap()
    yT = nc.dram_tensor("yT", [c, ntok], f32, kind="ExternalOutput").ap()

    cbuf = nc.dram_tensor("cbuf", [e, ntok], f32).ap()    # expert candidates
    gbuf = nc.dram_tensor("gbuf", [2, ntok], f32).ap()    # rank gate rows
    slotbuf = nc.dram_tensor("slotbuf", [2, ntok], f32).ap()  # rank -> slots

    with tile.TileContext(nc) as tc:
      for rep in range(repeat):
       with ExitStack() as ctx:
             cpool = ctx.enter_context(tc.tile_pool(name=f"const{rep}", bufs=1))
             xtp = ctx.enter_context(tc.tile_pool(name=f"xt{rep}", bufs=1))
             ctp = ctx.enter_context(tc.tile_pool(name=f"contrib{rep}", bufs=1))
             idxp = ctx.enter_context(tc.tile_pool(name=f"idx{rep}", bufs=1))

             # ---- constants ----
             gwsb = cpool.tile([P, CK * e], f32)
             nc.sync.dma_start(gwsb[:].rearrange("p (k e) -> p k e", e=e),
                               gw.rearrange("(k p) e -> p k e", p=P))
             trisb = cpool.tile([P, P], f32)
             nc.sync.dma_start(trisb[:], tri)
             id8sb = cpool.tile([e, e], f32)
             nc.sync.dma_start(id8sb[:], id8)
             gbsb = cpool.tile([e, 1], f32)
             nc.sync.dma_start(gbsb[:], gb)
             ebsb = cpool.tile([1, e], f32)
             nc.sync.dma_start(ebsb[:], ebase)
             ones1 = cpool.tile([1, P], f32)
             nc.vector.memset(ones1[:], 1.0)
             onescol = cpool.tile([P, 1], f32)
             nc.vector.memset(onescol[:], 1.0)
             iotaE_i = cpool.tile([P, TCH * e], dt.int32)
             nc.gpsimd.iota(iotaE_i[:], pattern=[[0, TCH], [1, e]], base=0,
                            channel_multiplier=0)
             iotaE = cpool.tile([P, TCH * e], f32)
             nc.vector.tensor_copy(iotaE[:], iotaE_i[:])
             toks_i = cpool.tile([P, TCH], dt.int32)
             nc.gpsimd.iota(toks_i[:], pattern=[[P, TCH]], base=0,
                            channel_multiplier=1)
             toksf = cpool.tile([P, TCH], f32)
             nc.vector.tensor_copy(toksf[:], toks_i[:])

             # ---- resident x (C-major) ----
             xts = []
             for k in range(CK):
                 t = xtp.tile([P, ntok], f32, tag=f"xt{k}")
                 nc.sync.dma_start(t[:], xT[k * P:(k + 1) * P, :])
                 xts.append(t)

             # contrib buffers (expert outputs, gate-scaled, slot-major, C-major)
             contrib = ([ctp.tile([P, NSLOT], f32, tag=f"cb{cc}", name=f"cb{cc}_{rep}")
                         for cc in range(CK)]
                        if level not in (40, 41, 42) else None)

             # combine gather indices (built in stage A)
             idx_all = [idxp.tile([P, ntok // 16], dt.int16, tag=f"ix{r}",
                                  name=f"ix{r}") for r in range(2)]

             # =============== Stage A: gate + routing ===============
             with tc.tile_pool(name=f"sa{rep}", bufs=2) as sa, \
                  tc.tile_pool(name=f"sa1{rep}", bufs=1) as sa1, \
                  tc.tile_pool(name=f"pa{rep}", bufs=1, space="PSUM") as pa, \
                  tc.tile_pool(name=f"pb{rep}", bufs=1, space="PSUM") as pb:
                 # gate logits, expert-major: lgT[e, tok].
                 # True fp32 matmul: fp32r is reduced-precision on HW and would
                 # flip top-2 picks vs the reference on near-ties.
                 lgT = sa1.tile([e, ntok], f32)
                 lgps = [pa.tile([e, 512], f32, space="PSUM", tag=f"lg{nh}",
                                 name=f"lg{nh}") for nh in range(NH)]
                 for k in range(CK):
                     for nh in range(NH):
                         nc.tensor.matmul(lgps[nh][:],
                                          lhsT=gwsb[:, k * e:(k + 1) * e],
                                          rhs=xts[k][:, nh * 512:(nh + 1) * 512],
                                          start=(k == 0), stop=(k == CK - 1))
                 for nh in range(NH):
                     nc.vector.tensor_scalar_add(lgT[:, nh * 512:(nh + 1) * 512],
                                                 lgps[nh][:], gbsb[:, :1])
                 # transpose to token-major [128, TCH, e]
                 lg = sa1.tile([P, TCH, e], f32)
                 for t in range(TCH):
                     ps = pa.tile([P, e], f32, space="PSUM", tag="tp")
                     nc.tensor.transpose(ps[:], lgT[:, t * P:(t + 1) * P], id8sb[:])
                     nc.scalar.activation(lg[:, t, :], ps[:], AF.Copy)
                 # softmax over experts
                 mx = sa.tile([P, TCH], f32)
                 nc.vector.tensor_reduce(mx[:], lg[:], axis=AX.X, op=ALU.max)
                 xm = sa.tile([P, TCH, e], f32)
                 nc.vector.tensor_tensor(out=xm[:], in0=lg[:],
                                         in1=mx[:].to_broadcast([P, TCH, e]),
                                         op=ALU.subtract)
                 ex = sa.tile([P, TCH, e], f32)
                 nc.scalar.activation(ex[:], xm[:], AF.Exp)
                 sm = sa.tile([P, TCH], f32)
                 nc.vector.tensor_reduce(sm[:], ex[:], axis=AX.X, op=ALU.add)
                 rs = sa.tile([P, TCH], f32)
                 nc.vector.reciprocal(rs[:], sm[:])
                 probs = sa.tile([P, TCH, e], f32)
                 nc.vector.tensor_tensor(out=probs[:], in0=ex[:],
                                         in1=rs[:].to_broadcast([P, TCH, e]),
                                         op=ALU.mult)
                 # top-2 by logits (same order as by probs)
                 mig = sa1.tile([P, TCH, 8], dt.uint32)
                 for t in range(TCH):
                     mv = sa.tile([P, 8], f32, tag="mv")
                     nc.vector.max(mv[:], lg[:, t, :])
                     nc.vector.max_index(mig[:, t, :], mv[:], lg[:, t, :])
                 migf = sa1.tile([P, TCH, 8], f32)
                 nc.vector.tensor_copy(migf[:], mig[:])

                 A = []  # one-hot masks per rank [P, TCH, e]
                 g = []  # gate values per rank [P, TCH]
                 for r in range(2):
                     Ar = sa1.tile([P, TCH, e], f32, tag=f"A{r}")
                     nc.vector.tensor_tensor(
                         out=Ar[:], in0=migf[:, :, r:r + 1].to_broadcast([P, TCH, e]),
                         in1=iotaE[:].rearrange("p (t e) -> p t e", e=e),
                         op=ALU.is_equal)
                     gr = sa1.tile([P, TCH], f32, tag=f"g{r}")
                     tmp = sa.tile([P, TCH, e], f32, tag="gt")
                     nc.vector.tensor_tensor(out=tmp[:], in0=probs[:], in1=Ar[:],
                                             op=ALU.mult)
                     nc.vector.tensor_reduce(gr[:], tmp[:], axis=AX.X, op=ALU.add)
                     A.append(Ar)
                     g.append(gr)
                 M = sa1.tile([P, TCH, e], f32)
                 nc.vector.tensor_tensor(out=M[:], in0=A[0][:], in1=A[1][:],
                                         op=ALU.add)

                 if level < 1:
                     break
                 # per-chunk prefix sums along tokens + running carry
                 carry = sa1.tile([1, e], f32)
                 nc.vector.memset(carry[:], 0.0)
                 pssb = sa1.tile([P, TCH, e], f32)  # global slot+1 per (tok, e)
                 for t in range(TCH):
                     pf = pb.tile([P, e], f32, space="PSUM", tag="pf")
                     nc.tensor.matmul(pf[:], lhsT=trisb[:], rhs=M[:, t, :],
                                      start=True, stop=True)
                     bv = sa.tile([1, e], f32, tag="bv")
                     nc.vector.tensor_tensor(out=bv[:], in0=ebsb[:], in1=carry[:],
                                             op=ALU.add)
                     bb = pb.tile([P, e], f32, space="PSUM", tag="bb")
                     nc.tensor.matmul(bb[:], lhsT=ones1[:], rhs=bv[:],
                                      start=True, stop=True)
                     bbs = sa.tile([P, e], f32, tag="bbs")
                     nc.scalar.activation(bbs[:], bb[:], AF.Copy)
                     nc.vector.tensor_tensor(out=pssb[:, t, :], in0=pf[:],
                                             in1=bbs[:], op=ALU.add)
                     totps = pb.tile([1, e], f32, space="PSUM", tag="tt")
                     nc.tensor.matmul(totps[:], lhsT=onescol[:], rhs=M[:, t, :],
                                      start=True, stop=True)
                     nc.vector.tensor_tensor(out=carry[:], in0=carry[:],
                                             in1=totps[:], op=ALU.add)

                 slots_f = []
                 slots_i = []
                 for r in range(2):
                     sel = sa.tile([P, TCH, e], f32, tag="sel")
                     nc.vector.scalar_tensor_tensor(out=sel[:], in0=pssb[:],
                                                    scalar=-1.0, in1=A[r][:],
                                                    op0=ALU.add, op1=ALU.mult)
                     sf = sa1.tile([P, TCH], f32, tag=f"sf{r}")
                     nc.vector.tensor_reduce(sf[:], sel[:], axis=AX.X, op=ALU.add)
                     si = sa1.tile([P, TCH], dt.int32, tag=f"si{r}")
                     nc.vector.tensor_copy(si[:], sf[:])
                     slots_f.append(sf)
                     slots_i.append(si)
                     # store token-order slots for the combine gather
                     nc.sync.dma_start(
                         slotbuf[r, :].rearrange("(t p) -> p t", p=P), sf[:])

                 if level < 2:
                     break
                 # candidate token ids per expert: tok if expert in top-2 else -1
                 tokp1 = sa.tile([P, TCH], f32, tag="tokp1")
                 nc.vector.tensor_scalar_add(tokp1[:], toksf[:], 1.0)
                 candf = sa1.tile([P, TCH, e], f32)
                 nc.vector.tensor_tensor(
                     out=candf[:],
                     in0=tokp1[:].rearrange("p (t o) -> p t o", o=1)
                     .to_broadcast([P, TCH, e]),
                     in1=M[:], op=ALU.mult)
                 nc.vector.tensor_scalar_add(candf[:], candf[:], -1.0)
                 for ei in range(e):
                     nc.sync.dma_start(
                         cbuf[ei, :].rearrange("(t p) -> p t", p=P),
                         candf[:, :, ei])
                 for r in range(2):
                     nc.sync.dma_start(
                         gbuf[r, :].rearrange("(t p) -> p t", p=P), g[r][:])

                 if level < 3:
                     break
                 # combine gather indices in ap_gather wrap layout
                 for r in range(2):
                     tmpw = sa.tile([16, ntok // 16], f32, tag="tw")
                     nc.sync.dma_start(
                         tmpw[:], slotbuf[r, :].rearrange("(f p) -> p f", p=16))
                     nc.vector.tensor_copy(idx_all[r][0:16, :], tmpw[:])
                     for sz in (16, 32, 64):
                         nc.sync.dma_start(idx_all[r][sz:2 * sz, :],
                                           idx_all[r][0:sz, :])

             if level < 4:
                 break
             # =============== Stage B: expert MLP ===============
             with tc.tile_pool(name=f"mb{rep}", bufs=2) as mb, \
                  tc.tile_pool(name=f"w1p{rep}", bufs=6) as w1p, \
                  tc.tile_pool(name=f"w2p{rep}", bufs=6) as w2p, \
                  tc.tile_pool(name=f"xgp{rep}", bufs=3) as xgp, \
                  tc.tile_pool(name=f"hp{rep}", bufs=1) as hp, \
                  tc.tile_pool(name=f"p1{rep}", bufs=2, space="PSUM") as p1, \
                  tc.tile_pool(name=f"p2{rep}", bufs=1, space="PSUM") as p2, \
                  tc.tile_pool(name=f"pg{rep}", bufs=2, space="PSUM") as pg:
                 for ei in range(e):
                     se = int(S[ei])
                     b0 = int(base[ei])
                     b1e = mb.tile([P, HK], f32, tag="b1e")
                     nc.sync.dma_start(
                         b1e[:].rearrange("p (k o) -> p k o", o=1),
                         b1[ei].rearrange("(k p) one -> p k one", p=P))
                     b2e = mb.tile([P, CK], f32, tag="b2e")
                     nc.sync.dma_start(
                         b2e[:].rearrange("p (k o) -> p k o", o=1),
                         b2[ei].rearrange("(k p) one -> p k one", p=P))
                     cw = mb.tile([16, ntok // 16], f32, tag="cw")
                     nc.sync.dma_start(
                         cw[:], cbuf[ei, :].rearrange("(f p) -> p f", p=16))
                     tkf = mb.tile([16, se // 16], f32, tag="tkf")
                     nc.vector.memset(tkf[:], 0.0)
                     nfd = mb.tile([1, 1], dt.uint32, tag="nfd")
                     nc.gpsimd.sparse_gather(tkf[:], cw[:], num_found=nfd[:])
                     nc.vector.tensor_scalar_min(tkf[:], tkf[:], float(ntok - 1))
                     nc.vector.tensor_scalar_max(tkf[:], tkf[:], 0.0)
                     t16 = mb.tile([P, se // 16], dt.int16, tag="t16")
                     nc.vector.tensor_copy(t16[0:16, :], tkf[:])
                     for sz in (16, 32, 64):
                         nc.sync.dma_start(t16[sz:2 * sz, :], t16[0:sz, :])

                     for (woff, W) in _mlp_tiles(se):
                         iw = woff // 16
                         # gather x columns for this slot tile
                         xg = []
                         for k in range(CK):
                             xgf = xgp.tile([P, W], f32, tag=f"xgf{k}")
                             nc.gpsimd.ap_gather(
                                 xgf[:], xts[k][:], t16[:, iw:iw + W // 16],
                                 channels=P, num_elems=ntok, d=1, num_idxs=W)
                             xgt = xgp.tile([P, W], bf16, tag=f"xg{k}")
                             nc.vector.tensor_copy(xgt[:], xgf[:])
                             xg.append(xgt)
                         if level == 40:
                             for k in range(CK):
                                 nc.gpsimd.dma_start(yT[k * P:(k + 1) * P, 0:W],
                                                     xg[k][:])
                             continue
                         # layer 1
                         hs = []
                         for hk in range(HK):
                             wrow = w1p.tile([P, CK * P], bf16, tag="w1r")
                             nc.sync.dma_start(wrow[:], w1[ei, hk])
                             ps = p1.tile([P, W], f32, space="PSUM", tag="ps1")
                             for k in range(CK):
                                 nc.tensor.matmul(ps[:], lhsT=wrow[:, k * P:(k + 1) * P],
                                                  rhs=xg[k][:],
                                                  start=(k == 0), stop=(k == CK - 1))
                             ht = hp.tile([P, W], bf16, tag=f"h{hk}")
                             nc.scalar.activation(ht[:], ps[:], AF.Relu,
                                                  bias=b1e[:, hk:hk + 1])
                             hs.append(ht)
                         if level == 41:
                             for hk in range(HK):
                                 nc.gpsimd.dma_start(
                                     yT[(hk % CK) * P:(hk % CK + 1) * P,
                                        (hk // CK) * W:(hk // CK) * W + W],
                                     hs[hk][:])
                             continue
                         if level == 42:
                             nc.sync.dma_start(yT[0:P, 0:W], gbc[:])
                             for hk in range(HK):
                                 nc.gpsimd.dma_start(
                                     yT[(hk % CK) * P:(hk % CK + 1) * P,
                                        (hk // CK) * W:(hk // CK) * W + W],
                                     hs[hk][:])
                             continue
                         # layer 2 in phases of up to 4 output chunks
                         for ch in range((CK + 3) // 4):
                             ncc = min(4, CK - ch * 4)
                             pss = [p2.tile([P, W], f32, space="PSUM",
                                            tag=f"ps2_{j}", name=f"ps2_{j}_{rep}")
                                    for j in range(ncc)]
                             for hk in range(HK):
                                 w2t = w2p.tile([P, ncc * P], bf16, tag="w2t")
                                 nc.sync.dma_start(
                                     w2t[:, :ncc * P],
                                     w2[ei, hk * P:(hk + 1) * P,
                                        ch * 4 * P:(ch * 4 + ncc) * P])
                                 for j in range(ncc):
                                     nc.tensor.matmul(
                                         pss[j][:], lhsT=w2t[:, j * P:(j + 1) * P],
                                         rhs=hs[hk][:],
                                         start=(hk == 0), stop=(hk == HK - 1))
                             for j in range(ncc):
                                 cc = ch * 4 + j
                                 nc.vector.tensor_scalar_add(
                                     contrib[cc][:, b0 + woff:b0 + woff + W],
                                     pss[j][:], b2e[:, cc:cc + 1])

             if level < 5 or level in (40, 41, 42):
                 break
             # =============== Stage C: combine ===============
             with tc.tile_pool(name=f"cb{rep}", bufs=3) as cbp, \
                  tc.tile_pool(name=f"cg{rep}", bufs=1) as cgp, \
                  tc.tile_pool(name=f"cp{rep}", bufs=2, space="PSUM") as cpp:
                 gbc = []
                 for r in range(2):
                     grow = cgp.tile([1, ntok], f32, tag=f"gr{r}",
                                     name=f"gr{r}_{rep}")
                     nc.sync.dma_start(grow[:], gbuf[r:r + 1, :])
                     gbt = cgp.tile([P, ntok], f32, tag=f"gb{r}",
                                    name=f"gb{r}_{rep}")
                     for nh in range(NH):
                         gps = cpp.tile([P, 512], f32, space="PSUM", tag="gps")
                         nc.tensor.matmul(gps[:], lhsT=ones1[:],
                                          rhs=grow[:, nh * 512:(nh + 1) * 512],
                                          start=True, stop=True)
                         nc.scalar.activation(gbt[:, nh * 512:(nh + 1) * 512],
                                              gps[:], AF.Copy)
                     gbc.append(gbt)
                 for cc in range(CK):
                     c1 = cbp.tile([P, ntok], f32, tag="c1")
                     c2 = cbp.tile([P, ntok], f32, tag="c2")
                     for r, ct in ((0, c1), (1, c2)):
                         nc.gpsimd.ap_gather(
                             ct[:], contrib[cc][:], idx_all[r][:],
                             channels=P, num_elems=NSLOT, d=1, num_idxs=ntok)
                     t1 = cbp.tile([P, ntok], f32, tag="t1")
                     nc.vector.tensor_tensor(out=t1[:], in0=c1[:],
                                             in1=gbc[0][:], op=ALU.mult)
                     ys = cbp.tile([P, ntok], f32, tag="ys")
                     nc.vector.scalar_tensor_tensor(
                         out=ys[:], in0=c2[:], scalar=1.0, in1=gbc[1][:],
                         op0=ALU.mult, op1=ALU.mult)
                     nc.vector.tensor_tensor(out=ys[:], in0=ys[:], in1=t1[:],
                                             op=ALU.add)
                     nc.sync.dma_start(yT[cc * P:(cc + 1) * P, :], ys[:])

    return nc


# ---------------- host side ----------------

def _host_caps(xf, gate_w, gate_b, ntok=NTOK, margin=16):
    """Slot capacities per expert from a host replica of the routing."""
    logits = xf.astype(np.float32) @ gate_w.astype(np.float32) + gate_b
    order = np.argpartition(-logits, TOPK - 1, axis=1)[:, :TOPK]
    ncore = xf.shape[0] // ntok
    counts = np.zeros((ncore, E), np.int64)
    for cc in range(ncore):
        sl = order[cc * ntok:(cc + 1) * ntok]
        counts[cc] = np.bincount(sl.ravel(), minlength=E)
    maxc = counts.max(axis=0)
    S = ((maxc + margin + 15) // 16) * 16
    pad = (-int(S.sum())) % P
    S[-1] += pad
    return S.astype(np.int64)


def kernel(x, gate_w, gate_b, w1, b1, w2, b2):
    from concourse.bass_utils import run_bass_kernel_spmd

    x = np.asarray(x, np.float32)
    gate_w = np.asarray(gate_w, np.float32)
    gate_b = np.asarray(gate_b, np.float32)
    import ml_dtypes
    w1 = np.asarray(w1, np.float32)
    e_, c_, h_ = w1.shape
    w1 = np.ascontiguousarray(
        w1.reshape(e_, c_ // P, P, h_ // P, P).transpose(0, 3, 2, 1, 4)
        .reshape(e_, h_ // P, P, c_).astype(ml_dtypes.bfloat16))
    b1 = np.asarray(b1, np.float32)
    w2 = np.ascontiguousarray(np.asarray(w2).astype(ml_dtypes.bfloat16))
    b2 = np.asarray(b2, np.float32)

    b, t, c = x.shape
    xf = x.reshape(b * t, c)
    S = _host_caps(xf, gate_w, gate_b)
    nc = build_program(S)

    ebase = np.concatenate([[0], np.cumsum(S)[:-1]]).astype(np.float32)
    shared = {
        "gw": gate_w,
        "gb": gate_b.reshape(E, 1).copy(),
        "w1": w1,
        "b1": b1.reshape(E, H, 1).copy(),
        "w2": w2,
        "b2": b2.reshape(E, C, 1).copy(),
        "tri": np.triu(np.ones((P, P), np.float32)),
        "id8": np.eye(E, dtype=np.float32),
        "ebase": ebase.reshape(1, E),
    }
    in_maps = []
    for cc in range(NCORE):
        sl = xf[cc * NTOK:(cc + 1) * NTOK]
        m = dict(shared)
        m["xT"] = np.ascontiguousarray(sl.T)
        in_maps.append(m)

    global LAST_BUILD, LAST_S
    LAST_BUILD = (nc, in_maps)
    LAST_S = S
    res = run_bass_kernel_spmd(nc, in_maps, core_ids=list(range(NCORE)))
    outs = [np.ascontiguousarray(r["yT"].T) for r in res.results]
    y = np.concatenate(outs, axis=0).reshape(b, t, c)
    return y.astype(np.float32)



# revision 18
# speedup vs baseline: 2.6251x; 2.6251x over previous
"""Trainium2 Bass kernel for top-2 MoE (nn_ExpertMemory).

Model (reference semantics):
    logits = x @ gate_w + gate_b          # (N, E)
    probs  = softmax(logits)
    gates, idx = top_k(probs, 2)
    out[n] = sum_k gates[n,k] * (relu(x[n] @ w1[e] + b1[e]) @ w2[e] + b2[e]),
             e = idx[n,k]
(The reference runs every expert densely, but combine weights are zero off
the top-2, so routed computation is mathematically identical.)

Strategy: data-parallel over tokens across 8 NeuronCores (1024 tokens each).
Each core, fully on device:
  1. gate matmul (true fp32) + softmax + top-2 on its tokens
  2. gates folded into the activations: x2[2t+r] = [g_r(t)*x[t], g_r(t), 0...]
     (bf16, 1152 features) written to DRAM; w1 augmented with a b1 row so
     layer 1 computes g*(x@w1 + b1) with no per-slot bias
  3. per-expert token lists via sparse_gather over candidate encodings
     (sentinel-filled to capacity, so no pad-value edge cases)
  4. dispatch via dma_gather(transpose=True): token rows gathered from DRAM
     directly into C-major SBUF layout (no gpsimd ap_gather, no transposes)
  5. layer 1 slot-moving (out [h, slots]); layer 2 with h as the stationary
     operand so the output lands token-major [slots, C] in PSUM
  6. combine via dma_scatter_add of bf16 rows into yT, which is
     pre-initialized with the gate-weighted b2 correction
     (sum_r g_r*b2[e_r]) computed by a tiny matmul.
Slot capacities are specialized per run from a host-side replica of the
routing (inputs only, margin 16); the device computes everything itself.
"""

import numpy as np
from contextlib import ExitStack

import concourse.bass as bass
import concourse.tile as tile
import concourse.mybir as mybir
from concourse import bacc

dt = mybir.dt
AF = mybir.ActivationFunctionType
ALU = mybir.AluOpType
AX = mybir.AxisListType

P = 128

# problem constants
B, T, C, E, H, TOPK = 4, 2048, 1024, 8, 2048, 2
NCORE = 8
NTOK = B * T // NCORE  # tokens per core
TCH = NTOK // P        # token chunks (8)
CK = C // P            # C chunks (8)
HK = H // P            # H chunks (16)
KA = CK + 1            # augmented contraction chunks (x | g | pad)
CF = KA * P            # x2 row length (1152)
NH = NTOK // 512       # 512-wide token halves for the gate matmul


def _tiles(s):
    """Split slot range s into moving tiles: full 512s, then the remainder
    (16-aligned). Tile starts are 128-aligned so L2 sub-tiles line up with
    the global slot chunks."""
    out = []
    off = 0
    rem = s
    while rem > 512:
        out.append((off, 512))
        off += 512
        rem -= 512
    if rem:
        out.append((off, rem))
    return out


def build_program(S, ntok=NTOK, level=9, repeat=1):
    nc = _build(S, ntok=ntok, level=level, repeat=repeat)
    nc.compile()
    return nc


def _build(S, ntok=NTOK, level=9, repeat=1):
    """S: per-expert slot capacities (multiples of 16, each <= 512)."""
    S = [int(s) for s in S]
    assert all(s % 16 == 0 and 16 <= s <= 512 for s in S)
    S128 = [(s + 127) // 128 * 128 for s in S]

    nc = bacc.Bacc("TRN2", target_bir_lowering=False, debug=False)

    f32, bf16 = dt.float32, dt.bfloat16
    xT = nc.dram_tensor("xT", [C, ntok], f32, kind="ExternalInput").ap()
    xtm = nc.dram_tensor("xtm", [ntok, C], bf16, kind="ExternalInput").ap()
    gw = nc.dram_tensor("gw", [C, E], f32, kind="ExternalInput").ap()
    gb = nc.dram_tensor("gb", [E, 1], f32, kind="ExternalInput").ap()
    w1a = nc.dram_tensor("w1a", [E, HK, P, KA * P], bf16,
                         kind="ExternalInput").ap()
    w2 = nc.dram_tensor("w2", [E, H, C], bf16, kind="ExternalInput").ap()
    b2e = nc.dram_tensor("b2e", [E, C], f32, kind="ExternalInput").ap()
    id8 = nc.dram_tensor("id8", [E, E], f32, kind="ExternalInput").ap()
    id128 = nc.dram_tensor("id128", [P, P], f32, kind="ExternalInput").ap()
    # +16 rows: trash target for pad-slot scatter writes (their payload is
    # zero, but pointing them at real rows would race the real adds within
    # the same scatter DMA — CCE read-modify-write is not atomic)
    yT = nc.dram_tensor("yT", [ntok + 16, C], bf16, kind="ExternalOutput").ap()

    cbufG = nc.dram_tensor("cbufG", [E, ntok], f32).ap()  # 2*tok+r+1 | -1
    cbufT = nc.dram_tensor("cbufT", [E, ntok], f32).ap()  # tok+1 | -1
    # scaled tokens; row 0 is all-zero so pad slots compute exact zeros
    x2 = nc.dram_tensor("x2", [2 * ntok + 16, CF], bf16).ap()

    with tile.TileContext(nc) as tc:
      for rep in range(repeat):
       with ExitStack() as ctx:
        cpool = ctx.enter_context(tc.tile_pool(name=f"const{rep}", bufs=1))

        # ---- constants ----
        gwsb = cpool.tile([P, CK * E], f32)
        nc.sync.dma_start(gwsb[:].rearrange("p (k e) -> p k e", e=E),
                          gw.rearrange("(k p) e -> p k e", p=P))
        id8sb = cpool.tile([E, E], f32)
        nc.sync.dma_start(id8sb[:], id8)
        id128sb = cpool.tile([P, P], f32)
        nc.sync.dma_start(id128sb[:], id128)
        gbsb = cpool.tile([E, 1], f32)
        nc.sync.dma_start(gbsb[:], gb)
        b2sb = cpool.tile([E, C], f32)
        nc.sync.dma_start(b2sb[:], b2e)
        iotaE_i = cpool.tile([P, TCH * E], dt.int32)
        nc.gpsimd.iota(iotaE_i[:], pattern=[[0, TCH], [1, E]], base=0,
                       channel_multiplier=0)
        iotaE = cpool.tile([P, TCH * E], f32)
        nc.vector.tensor_copy(iotaE[:], iotaE_i[:])
        toks_i = cpool.tile([P, TCH], dt.int32)
        nc.gpsimd.iota(toks_i[:], pattern=[[P, TCH]], base=0,
                       channel_multiplier=1)
        toksf = cpool.tile([P, TCH], f32)
        nc.vector.tensor_copy(toksf[:], toks_i[:])
        slotio_i = cpool.tile([16, 512 // 16], dt.int32)
        nc.gpsimd.iota(slotio_i[:], pattern=[[16, 512 // 16]], base=0,
                       channel_multiplier=1)
        slotio = cpool.tile([16, 512 // 16], f32)
        nc.vector.tensor_copy(slotio[:], slotio_i[:])
        ones16 = cpool.tile([P, 16], f32)
        nc.vector.memset(ones16[:], 1.0)

        # per-rank gates in token order, kept for later stages
        gpool = ctx.enter_context(tc.tile_pool(name=f"gk{rep}", bufs=1))
        g = []

        # =============== Stage A: gate + routing metadata ===============
        with tc.tile_pool(name=f"sa{rep}", bufs=2) as sa, \
             tc.tile_pool(name=f"sa1{rep}", bufs=1) as sa1, \
             tc.tile_pool(name=f"xt{rep}", bufs=1) as xtp, \
             tc.tile_pool(name=f"pa{rep}", bufs=1, space="PSUM") as pa, \
             tc.tile_pool(name=f"pb{rep}", bufs=1, space="PSUM") as pb:
            # resident x (C-major, fp32) for the gate matmul
            xts = []
            for k in range(CK):
                t = xtp.tile([P, ntok], f32, tag=f"xt{k}")
                nc.sync.dma_start(t[:], xT[k * P:(k + 1) * P, :])
                xts.append(t)
            # resident x (token-major, bf16) for the x2 build
            xtmsb = xtp.tile([P, TCH, C], bf16, tag="xtm")
            nc.sync.dma_start(xtmsb[:],
                              xtm.rearrange("(tch p) c -> p tch c", p=P))

            # gate logits, expert-major: lgT[e, tok]. True fp32 matmul:
            # fp32r is reduced-precision on HW and would flip top-2 picks.
            lgT = sa1.tile([E, ntok], f32)
            lgps = [pa.tile([E, 512], f32, space="PSUM", tag=f"lg{nh}",
                            name=f"lg{nh}_{rep}") for nh in range(NH)]
            for k in range(CK):
                for nh in range(NH):
                    nc.tensor.matmul(lgps[nh][:],
                                     lhsT=gwsb[:, k * E:(k + 1) * E],
                                     rhs=xts[k][:, nh * 512:(nh + 1) * 512],
                                     start=(k == 0), stop=(k == CK - 1))
            for nh in range(NH):
                nc.vector.tensor_scalar_add(lgT[:, nh * 512:(nh + 1) * 512],
                                            lgps[nh][:], gbsb[:, :1])
            # transpose to token-major [128, TCH, e]
            lg = sa1.tile([P, TCH, E], f32)
            for t in range(TCH):
                ps = pa.tile([P, E], f32, space="PSUM", tag="tp")
                nc.tensor.transpose(ps[:], lgT[:, t * P:(t + 1) * P], id8sb[:])
                nc.scalar.activation(lg[:, t, :], ps[:], AF.Copy)
            # softmax over experts
            mx = sa.tile([P, TCH], f32)
            nc.vector.tensor_reduce(mx[:], lg[:], axis=AX.X, op=ALU.max)
            xm = sa.tile([P, TCH, E], f32)
            nc.vector.tensor_tensor(out=xm[:], in0=lg[:],
                                    in1=mx[:].to_broadcast([P, TCH, E]),
                                    op=ALU.subtract)
            ex = sa.tile([P, TCH, E], f32)
            nc.scalar.activation(ex[:], xm[:], AF.Exp)
            sm = sa.tile([P, TCH], f32)
            nc.vector.tensor_reduce(sm[:], ex[:], axis=AX.X, op=ALU.add)
            rs = sa.tile([P, TCH], f32)
            nc.vector.reciprocal(rs[:], sm[:])
            probs = sa.tile([P, TCH, E], f32)
            nc.vector.tensor_tensor(out=probs[:], in0=ex[:],
                                    in1=rs[:].to_broadcast([P, TCH, E]),
                                    op=ALU.mult)
            # top-2 by logits (same order as by probs)
            mig = sa1.tile([P, TCH, 8], dt.uint32)
            for t in range(TCH):
                mv = sa.tile([P, 8], f32, tag="mv")
                nc.vector.max(mv[:], lg[:, t, :])
                nc.vector.max_index(mig[:, t, :], mv[:], lg[:, t, :])
            migf = sa1.tile([P, TCH, 8], f32)
            nc.vector.tensor_copy(migf[:], mig[:])

            A = []  # one-hot masks per rank [P, TCH, e]
            for r in range(2):
                Ar = sa1.tile([P, TCH, E], f32, tag=f"A{r}")
                nc.vector.tensor_tensor(
                    out=Ar[:], in0=migf[:, :, r:r + 1].to_broadcast([P, TCH, E]),
                    in1=iotaE[:].rearrange("p (t e) -> p t e", e=E),
                    op=ALU.is_equal)
                gr = gpool.tile([P, TCH], f32, tag=f"g{r}")
                tmp = sa.tile([P, TCH, E], f32, tag="gt")
                nc.vector.tensor_tensor(out=tmp[:], in0=probs[:], in1=Ar[:],
                                        op=ALU.mult)
                nc.vector.tensor_reduce(gr[:], tmp[:], axis=AX.X, op=ALU.add)
                A.append(Ar)
                g.append(gr)
            M = sa1.tile([P, TCH, E], f32)
            nc.vector.tensor_tensor(out=M[:], in0=A[0][:], in1=A[1][:],
                                    op=ALU.add)

            # per-expert token counts, replicated on 16 partitions (used to
            # mask off sparse_gather's junk tail beyond the found count)
            Mre = sa.tile([P, E, TCH], f32, tag="Mre")
            nc.vector.tensor_copy(Mre[:], M[:].rearrange("p t e -> p e t"))
            cntp = pb.tile([16, E * TCH], f32, space="PSUM", tag="cntp")
            nc.tensor.matmul(cntp[:], lhsT=ones16[:],
                             rhs=Mre[:].rearrange("p e t -> p (e t)"),
                             start=True, stop=True)
            cntet = sa.tile([16, E, TCH], f32, tag="cntet")
            nc.scalar.activation(cntet[:],
                                 cntp[:].rearrange("p (e t) -> p e t", e=E),
                                 AF.Copy)
            cnt16 = gpool.tile([16, E], f32, tag="cnt16")
            nc.vector.tensor_reduce(cnt16[:], cntet[:], axis=AX.X, op=ALU.add)

            if level < 1:
                break
            # candidate encodings (+1-shifted so sparse-gather pads, which
            # are <= 0, can be clamped to the zero row / token 0):
            #   G = 2*tok + r + 1 (else -1), T = tok + 1 (else -1)
            tokp2 = sa.tile([P, TCH], f32, tag="tokp2")
            nc.vector.tensor_scalar_add(tokp2[:], toksf[:], 2.0)
            tok2 = sa.tile([P, TCH], f32, tag="tok2")
            nc.vector.tensor_scalar(tok2[:], toksf[:], 2.0, 2.0,
                                    op0=ALU.mult, op1=ALU.add)
            candG = sa1.tile([P, TCH, E], f32)
            nc.vector.tensor_tensor(
                out=candG[:], in0=tok2[:].to_broadcast([P, TCH, E]),
                in1=M[:], op=ALU.mult)
            nc.vector.tensor_tensor(out=candG[:], in0=candG[:], in1=A[1][:],
                                    op=ALU.add)
            nc.vector.tensor_scalar_add(candG[:], candG[:], -1.0)
            candT = sa1.tile([P, TCH, E], f32)
            nc.vector.tensor_tensor(
                out=candT[:], in0=tokp2[:].to_broadcast([P, TCH, E]),
                in1=M[:], op=ALU.mult)
            nc.vector.tensor_scalar_add(candT[:], candT[:], -1.0)
            for ei in range(E):
                nc.sync.dma_start(
                    cbufG[ei, :].rearrange("(t p) -> p t", p=P),
                    candG[:, :, ei])
                nc.sync.dma_start(
                    cbufT[ei, :].rearrange("(t p) -> p t", p=P),
                    candT[:, :, ei])

            # ---- x2: gate-scaled tokens, bf16, rows 1 + 2t + r ----
            zrow = sa.tile([1, CF], bf16, tag="zrow")
            nc.vector.memset(zrow[:], 0.0)
            nc.sync.dma_start(x2[0:1, :], zrow[:])
            for r in range(2):
                g16 = sa.tile([P, TCH], bf16, tag="g16")
                nc.vector.tensor_copy(g16[:], g[r][:])
                sc = sa1.tile([P, TCH, CF], bf16, tag=f"sc{r}")
                nc.vector.memset(sc[:, :, C:], 0.0)
                nc.vector.tensor_tensor(
                    out=sc[:, :, 0:C], in0=xtmsb[:],
                    in1=g16[:].to_broadcast([P, TCH, C]), op=ALU.mult)
                nc.vector.tensor_copy(
                    sc[:, :, C:C + 1],
                    g[r][:].rearrange("p (t o) -> p t o", o=1))
                nc.sync.dma_start(
                    x2[1:1 + 2 * ntok, :].rearrange(
                        "(tch p two) f -> p tch two f",
                        p=P, two=2)[:, :, r, :],
                    sc[:])

            # ---- yT init: sum_r g_r * b2[e_r] (+ zeroed trash rows) ----
            wtok = sa1.tile([P, TCH, E], f32, tag="wtok")
            nc.vector.tensor_tensor(
                out=wtok[:], in0=A[0][:],
                in1=g[0][:].to_broadcast([P, TCH, E]), op=ALU.mult)
            wtk1 = sa.tile([P, TCH, E], f32, tag="wtk1")
            nc.vector.tensor_tensor(
                out=wtk1[:], in0=A[1][:],
                in1=g[1][:].to_broadcast([P, TCH, E]), op=ALU.mult)
            nc.vector.tensor_tensor(out=wtok[:], in0=wtok[:], in1=wtk1[:],
                                    op=ALU.add)
            wTe = sa1.tile([E, TCH * P], f32, tag="wTe")
            for t in range(TCH):
                pw = pb.tile([E, P], f32, space="PSUM", tag="pw")
                nc.tensor.transpose(pw[:], wtok[:, t, :], id128sb[:])
                nc.scalar.activation(wTe[:, t * P:(t + 1) * P], pw[:], AF.Copy)
            ycorr = sa1.tile([P, TCH, C], bf16, tag="ycorr")
            for t in range(TCH):
                for hh in range(2):
                    pc = pb.tile([P, 512], f32, space="PSUM", tag="pc")
                    nc.tensor.matmul(pc[:], lhsT=wTe[:, t * P:(t + 1) * P],
                                     rhs=b2sb[:, hh * 512:(hh + 1) * 512],
                                     start=True, stop=True)
                    nc.scalar.activation(ycorr[:, t, hh * 512:(hh + 1) * 512],
                                         pc[:], AF.Copy)
            nc.sync.dma_start(
                yT[0:ntok, :].rearrange("(tch p) c -> p tch c", p=P),
                ycorr[:])

        if level < 2:
            break
        # =============== Stage B: expert MLP + scatter ===============
        with tc.tile_pool(name=f"mb{rep}", bufs=2) as mb, \
             tc.tile_pool(name=f"w1p{rep}", bufs=6) as w1p, \
             tc.tile_pool(name=f"w2p{rep}", bufs=2) as w2p, \
             tc.tile_pool(name=f"xgp{rep}", bufs=2) as xgp, \
             tc.tile_pool(name=f"hp{rep}", bufs=2) as hp, \
             tc.tile_pool(name=f"ystp{rep}", bufs=2) as ystp, \
             tc.tile_pool(name=f"p1{rep}", bufs=2, space="PSUM") as p1, \
             tc.tile_pool(name=f"p2{rep}", bufs=2, space="PSUM") as p2:
            for ei in range(E):
                Se = S[ei]
                Sg = S128[ei]
                nsub = (Se + 127) // 128
                # ---- token lists ----
                # pads are <= 0 after sparse_gather (we pre-fill -1, interp
                # pads -1): gather pads clamp to the zero row 0, so pad
                # slots compute exact-zero contributions and their scatter
                # destination (token 0) is harmless.
                cwG = mb.tile([16, ntok // 16], f32, tag="cwG")
                nc.sync.dma_start(
                    cwG[:], cbufG[ei, :].rearrange("(f p) -> p f", p=16))
                cwT = mb.tile([16, ntok // 16], f32, tag="cwT")
                nc.sync.dma_start(
                    cwT[:], cbufT[ei, :].rearrange("(f p) -> p f", p=16))

                tkG = mb.tile([16, Sg // 16], f32, tag="tkG")
                nc.vector.memset(tkG[:], -1.0)
                nfdG = mb.tile([1, 1], dt.uint32, tag="nfdG")
                nc.gpsimd.sparse_gather(tkG[:, :Se // 16], cwG[:],
                                        num_found=nfdG[:])
                gf = mb.tile([16, Sg // 16], f32, tag="gf")
                nc.vector.tensor_scalar(gf[:], tkG[:], 0.0, float(2 * ntok),
                                        op0=ALU.max, op1=ALU.min)
                # zero the junk tail: slots >= count gather the zero row
                msk = mb.tile([16, Sg // 16], f32, tag="msk")
                nc.vector.tensor_tensor(
                    out=msk[:], in0=slotio[:, :Sg // 16],
                    in1=cnt16[:, ei:ei + 1].to_broadcast([16, Sg // 16]),
                    op=ALU.is_lt)
                nc.vector.tensor_tensor(out=gf[:], in0=gf[:], in1=msk[:],
                                        op=ALU.mult)
                t16G = mb.tile([P, Sg // 16], dt.int16, tag="t16G")
                nc.vector.tensor_copy(t16G[0:16, :], gf[:])
                for sz in (16, 32, 64):
                    nc.sync.dma_start(t16G[sz:2 * sz, :], t16G[0:sz, :])

                tkT = mb.tile([16, Se // 16], f32, tag="tkT")
                nc.vector.memset(tkT[:], -1.0)
                nfdT = mb.tile([1, 1], dt.uint32, tag="nfdT")
                nc.gpsimd.sparse_gather(tkT[:], cwT[:], num_found=nfdT[:])
                stf = mb.tile([16, Se // 16], f32, tag="stf")
                nc.vector.tensor_scalar(stf[:], tkT[:], 1.0, float(ntok),
                                        op0=ALU.max, op1=ALU.min)
                nc.vector.tensor_scalar_add(stf[:], stf[:], -1.0)
                # divert junk-tail slots to the trash row `ntok`
                nc.vector.scalar_tensor_tensor(
                    out=stf[:], in0=stf[:], scalar=float(-ntok),
                    in1=msk[:, :Se // 16], op0=ALU.add, op1=ALU.mult)
                nc.vector.tensor_scalar_add(stf[:], stf[:], float(ntok))
                t16T = mb.tile([P, Se // 16], dt.int16, tag="t16T")
                nc.vector.tensor_copy(t16T[0:16, :], stf[:])
                for sz in (16, 32, 64):
                    nc.sync.dma_start(t16T[sz:2 * sz, :], t16T[0:sz, :])

                # ---- dispatch gather: C-major [128, KA, Sg] bf16 ----
                xg = xgp.tile([P, KA, Sg], bf16, tag="xg")
                nc.gpsimd.dma_gather(xg[:], x2, t16G[:], Sg, Sg, CF,
                                     transpose=True)

                # ---- w2 resident for this expert ----
                w2t = []
                for hk in range(HK):
                    wt = w2p.tile([P, C], bf16, tag=f"w2_{hk}")
                    nc.sync.dma_start(
                        wt[:], w2[ei, hk * P:(hk + 1) * P, :])
                    w2t.append(wt)

                yst = ystp.tile([P, nsub, C], bf16, tag="yst")
                if level < 3:
                    continue
                for (woff, W) in _tiles(Se):
                    # layer 1: out [h, slots]
                    hs = []
                    for hk in range(HK):
                        wrow = w1p.tile([P, KA * P], bf16, tag="w1r")
                        nc.sync.dma_start(wrow[:], w1a[ei, hk])
                        ps = p1.tile([P, W], f32, space="PSUM", tag="ps1")
                        for k in range(KA):
                            nc.tensor.matmul(
                                ps[:], lhsT=wrow[:, k * P:(k + 1) * P],
                                rhs=xg[:, k, woff:woff + W],
                                start=(k == 0), stop=(k == KA - 1))
                        ht = hp.tile([P, W], bf16, tag=f"h{hk}")
                        nc.scalar.activation(ht[:], ps[:], AF.Relu)
                        hs.append(ht)
                    if level < 4:
                        continue
                    # layer 2: h stationary -> out token-major [slots, C]
                    for sub in range((W + 127) // 128):
                        lo = sub * 128
                        wsub = min(128, W - lo)
                        gsub = (woff + lo) // 128
                        psA = p2.tile([P, 512], f32, space="PSUM", tag="ps2a")
                        psB = p2.tile([P, 512], f32, space="PSUM", tag="ps2b")
                        for hk in range(HK):
                            nc.tensor.matmul(
                                psA[0:wsub, :],
                                lhsT=hs[hk][:, lo:lo + wsub],
                                rhs=w2t[hk][:, 0:512],
                                start=(hk == 0), stop=(hk == HK - 1))
                            nc.tensor.matmul(
                                psB[0:wsub, :],
                                lhsT=hs[hk][:, lo:lo + wsub],
                                rhs=w2t[hk][:, 512:1024],
                                start=(hk == 0), stop=(hk == HK - 1))
                        nc.scalar.activation(yst[0:wsub, gsub, 0:512],
                                             psA[0:wsub, :], AF.Copy)
                        nc.scalar.activation(yst[0:wsub, gsub, 512:1024],
                                             psB[0:wsub, :], AF.Copy)
                if level < 5:
                    continue
                # ---- combine: scatter-add token rows into yT ----
                nc.gpsimd.dma_scatter_add(yT, yst[:], t16T[:], Se, Se, C)

    return nc


# ---------------- host side ----------------

def _host_caps(xf, gate_w, gate_b, ntok=NTOK, margin=16):
    """Slot capacities per expert from a host replica of the routing."""
    logits = xf.astype(np.float32) @ gate_w.astype(np.float32) + gate_b
    order = np.argpartition(-logits, TOPK - 1, axis=1)[:, :TOPK]
    ncore = xf.shape[0] // ntok
    counts = np.zeros((ncore, E), np.int64)
    for cc in range(ncore):
        sl = order[cc * ntok:(cc + 1) * ntok]
        counts[cc] = np.bincount(sl.ravel(), minlength=E)
    maxc = counts.max(axis=0)
    S = ((maxc + margin + 15) // 16) * 16
    assert S.max() <= 512, f"capacity overflow: {S}"
    return S.astype(np.int64)


def kernel(x, gate_w, gate_b, w1, b1, w2, b2):
    from concourse.bass_utils import run_bass_kernel_spmd
    import ml_dtypes

    x = np.asarray(x, np.float32)
    gate_w = np.asarray(gate_w, np.float32)
    gate_b = np.asarray(gate_b, np.float32)
    w1 = np.asarray(w1, np.float32)
    b1 = np.asarray(b1, np.float32)
    w2 = np.asarray(w2, np.float32)
    b2 = np.asarray(b2, np.float32)

    # augmented w1: [E, HK, P, KA*P]; chunk CK row 0 carries b1
    w1r = (w1.reshape(E, CK, P, HK, P).transpose(0, 3, 2, 1, 4)
           .reshape(E, HK, P, C))
    w1aug = np.zeros((E, HK, P, KA * P), np.float32)
    w1aug[..., :C] = w1r
    w1aug[:, :, 0, C:C + P] = b1.reshape(E, HK, P)
    w1aug = np.ascontiguousarray(w1aug.astype(ml_dtypes.bfloat16))
    w2b = np.ascontiguousarray(w2.astype(ml_dtypes.bfloat16))

    b, t, c = x.shape
    xf = x.reshape(b * t, c)
    S = _host_caps(xf, gate_w, gate_b)
    nc = build_program(S)

    shared = {
        "gw": gate_w,
        "gb": gate_b.reshape(E, 1).copy(),
        "w1a": w1aug,
        "w2": w2b,
        "b2e": b2,
        "id8": np.eye(E, dtype=np.float32),
        "id128": np.eye(P, dtype=np.float32),
    }
    in_maps = []
    for cc in range(NCORE):
        sl = xf[cc * NTOK:(cc + 1) * NTOK]
        m = dict(shared)
        m["xT"] = np.ascontiguousarray(sl.T)
        m["xtm"] = np.ascontiguousarray(sl.astype(ml_dtypes.bfloat16))
        in_maps.append(m)

    global LAST_BUILD, LAST_S
    LAST_BUILD = (nc, in_maps)
    LAST_S = S
    res = run_bass_kernel_spmd(nc, in_maps, core_ids=list(range(NCORE)))
    outs = [np.asarray(r["yT"][:NTOK]).astype(np.float32)
            for r in res.results]
    y = np.concatenate(outs, axis=0).reshape(b, t, c)
    return y
